# revision 13
# baseline (speedup 1.0000x reference)
"""Trainium2 Bass kernel: causal conv1d decode-step with conv-state cache update.

Problem (full shapes): x (8192, 1, 4096) f32, conv_state (16384, 3, 4096) f32,
weight (4, 4096) f32, bias (4096,) f32, conv_state_indices (8192,) int32.

Returns (out, new_conv_state):
  out[b, 0, :]        = silu(bias + sum_k w[k] * xnew[b, k, :])
  new_state[idx[b]]   = [cs1, cs2, x[b]]        (time-shifted tail)
  untouched slots pass through unchanged.

Sharding: batch rows (and their cache slots — indices are arange, so slot
ownership == batch shard) split across 8 NeuronCores; weight/bias replicated.
Each core streams its 1024 rows: conv-state row in (48KB), x row in (16KB),
out row (16KB) + shifted tail (48KB) back out — memory-bound by design.

Weights/bias are broadcast across the 128 SBUF partitions on-chip with a
ones-vector matmul on the (otherwise idle) tensor engine, so HBM sees only
the 80KB weight read instead of a 128x replicated read.
"""

import numpy as np

B = 8192
S = 1
D = 4096
W = 4
NUM_SLOTS = 16384
N_CORES = 8
BPC = B // N_CORES  # 1024 batch rows per core
P = 128  # SBUF partitions
FC = 2048  # free-dim (d) chunk per tile

_NC = None


def _build_nc(bpc=BPC, d=D, fc=FC, debug=False):
    from contextlib import ExitStack

    import concourse.bacc as bacc
    import concourse.tile as tile
    import concourse.mybir as mybir

    fp32 = mybir.dt.float32
    n_row = bpc // P
    n_col = d // fc
    seg = min(1024, fc)  # broadcast segment (psum tile = seg/512 banks)
    n_seg = fc // seg
    mm = min(512, seg)  # per-matmul free-dim limit

    nc = bacc.Bacc("TRN2", target_bir_lowering=False, debug=debug)

    x_d = nc.dram_tensor("x", [bpc, d], fp32, kind="ExternalInput")
    cs_d = nc.dram_tensor("cs", [bpc, 3, d], fp32, kind="ExternalInput")
    wb_d = nc.dram_tensor("wb", [1, 5 * d], fp32, kind="ExternalInput")
    out_d = nc.dram_tensor("out", [bpc, d], fp32, kind="ExternalOutput")
    tail_d = nc.dram_tensor("tail", [bpc, 3, d], fp32, kind="ExternalOutput")

    with tile.TileContext(nc) as tc, ExitStack() as ctx:
        constp = ctx.enter_context(tc.tile_pool(name="const", bufs=1))
        iop = ctx.enter_context(tc.tile_pool(name="io", bufs=2))
        outp = ctx.enter_context(tc.tile_pool(name="outp", bufs=3))
        accp = ctx.enter_context(tc.tile_pool(name="acc", bufs=2))
        tmpp = ctx.enter_context(tc.tile_pool(name="tmp", bufs=1))
        stripp = ctx.enter_context(tc.tile_pool(name="strip", bufs=2))
        psump = ctx.enter_context(tc.tile_pool(name="psum", bufs=4, space="PSUM"))

        # --- broadcast weights+bias to all 128 partitions via PE ---
        # ones[1,128].T @ wb_strip[1,N] -> psum[128,N], then ACT-copy to SBUF.
        ones_t = constp.tile([1, P], fp32)
        nc.vector.memset(ones_t[0:1, :], 1.0)
        wbt = constp.tile([P, 5 * d], fp32)
        for c in range(n_col):
            for k in range(5):
                for s in range(n_seg):
                    off = k * d + c * fc + s * seg
                    strip = stripp.tile([1, seg], fp32, tag="strip")
                    nc.sync.dma_start(strip[0:1, :], wb_d[0:1, off : off + seg])
                    pt = psump.tile([P, seg], fp32, tag="bcast")
                    for m in range(seg // mm):
                        nc.tensor.matmul(
                            pt[:, m * mm : (m + 1) * mm],
                            ones_t[0:1, :],
                            strip[0:1, m * mm : (m + 1) * mm],
                            start=True,
                            stop=True,
                        )
                    nc.scalar.copy(wbt[:, off : off + seg], pt[:, :])

        def wsl(k, c):
            return wbt[:, k * d + c * fc : k * d + (c + 1) * fc]

        # --- main streaming loop ---
        # c-outer: the c=1 weight load hides behind the entire c=0 sweep.
        for c in range(n_col):
            cols = slice(c * fc, (c + 1) * fc)
            for r in range(n_row):
                rows = slice(r * P, (r + 1) * P)
                # xnew tile: planes 0-2 = conv state, plane 3 = x
                xs_t = iop.tile([P, 4, fc], fp32, tag="xs")
                nc.sync.dma_start(xs_t[:, 0:3, :], cs_d[rows, :, cols])
                nc.sync.dma_start(xs_t[:, 3, :], x_d[rows, cols])

                acc = accp.tile([P, fc], fp32, tag="acc")
                tmp = tmpp.tile([P, fc], fp32, tag="tmp")
                nc.vector.tensor_mul(acc[:, :], xs_t[:, 0, :], wsl(0, c))
                nc.vector.tensor_mul(tmp[:, :], xs_t[:, 1, :], wsl(1, c))
                nc.vector.tensor_add(acc[:, :], acc[:, :], tmp[:, :])
                nc.vector.tensor_mul(tmp[:, :], xs_t[:, 2, :], wsl(2, c))
                nc.vector.tensor_add(acc[:, :], acc[:, :], tmp[:, :])
                nc.vector.tensor_mul(tmp[:, :], xs_t[:, 3, :], wsl(3, c))
                nc.vector.tensor_add(acc[:, :], acc[:, :], tmp[:, :])
                nc.vector.tensor_add(acc[:, :], acc[:, :], wsl(4, c))

                out_t = outp.tile([P, fc], fp32, tag="out")
                nc.scalar.activation(
                    out_t[:, :], acc[:, :], mybir.ActivationFunctionType.Silu
                )
                nc.sync.dma_start(out_d[rows, cols], out_t[:, :])
                nc.sync.dma_start(tail_d[rows, :, cols], xs_t[:, 1:4, :])

    nc.compile()
    return nc


def _get_nc():
    global _NC
    if _NC is None:
        _NC = _build_nc()
    return _NC


def run_spmd(in_maps, trace=False):
    from concourse import bass_utils

    return bass_utils.run_bass_kernel_spmd(
        _get_nc(), in_maps, core_ids=list(range(N_CORES)), trace=trace
    )


def make_wb(weight, bias, d=D, fc=FC):
    return (
        np.concatenate([weight, bias[None, :]], axis=0)
        .astype(np.float32)
        .reshape(1, 5 * d)
    )


def make_in_maps(x, cs_rows, weight, bias):
    wb = make_wb(weight, bias)
    in_maps = []
    for c in range(N_CORES):
        sl = slice(c * BPC, (c + 1) * BPC)
        in_maps.append(
            {
                "x": np.ascontiguousarray(x[sl, 0, :], dtype=np.float32),
                "cs": np.ascontiguousarray(cs_rows[sl], dtype=np.float32),
                "wb": wb,
            }
        )
    return in_maps


def kernel(x, conv_state, weight, bias, conv_state_indices, _trace=False):
    x = np.asarray(x, dtype=np.float32)
    conv_state = np.asarray(conv_state, dtype=np.float32)
    weight = np.asarray(weight, dtype=np.float32)
    bias = np.asarray(bias, dtype=np.float32)
    idx = np.asarray(conv_state_indices, dtype=np.int32)

    fast = bool(np.array_equal(idx, np.arange(B, dtype=np.int32)))
    cs_rows = conv_state[:B] if fast else conv_state[idx]

    res = run_spmd(make_in_maps(x, cs_rows, weight, bias), trace=_trace)
    results = res.results

    out = np.concatenate([np.asarray(r["out"]) for r in results], axis=0)
    out = out.reshape(B, S, D)
    tails = np.concatenate([np.asarray(r["tail"]) for r in results], axis=0)

    new_state = conv_state.copy()
    if fast:
        new_state[:B] = tails
    else:
        new_state[idx] = tails

    if _trace:
        return (out, new_state), res
    return out, new_state


# revision 14
# speedup vs baseline: 1.0347x; 1.0347x over previous
"""Trainium2 Bass kernel: causal conv1d decode-step with conv-state cache update.

Problem (full shapes): x (8192, 1, 4096) f32, conv_state (16384, 3, 4096) f32,
weight (4, 4096) f32, bias (4096,) f32, conv_state_indices (8192,) int32.

Returns (out, new_conv_state):
  out[b, 0, :]        = silu(bias + sum_k w[k] * xnew[b, k, :])
  new_state[idx[b]]   = [cs1, cs2, x[b]]        (time-shifted tail)
  untouched slots pass through unchanged.

Sharding: batch rows (and their cache slots — indices are arange, so slot
ownership == batch shard) split across 8 NeuronCores; weight/bias replicated.
Each core streams its 1024 rows: conv-state row in (48KB), x row in (16KB),
out row (16KB) + shifted tail (48KB) back out — memory-bound by design.

Weights/bias are broadcast across the 128 SBUF partitions on-chip with a
ones-vector matmul on the (otherwise idle) tensor engine, so HBM sees only
the 80KB weight read instead of a 128x replicated read.
"""

import numpy as np

B = 8192
S = 1
D = 4096
W = 4
NUM_SLOTS = 16384
N_CORES = 8
BPC = B // N_CORES  # 1024 batch rows per core
P = 128  # SBUF partitions
FC = 2048  # free-dim (d) chunk per tile

_NC = None


def _build_nc(bpc=BPC, d=D, fc=FC, debug=False):
    from contextlib import ExitStack

    import concourse.bacc as bacc
    import concourse.tile as tile
    import concourse.mybir as mybir

    fp32 = mybir.dt.float32
    n_row = bpc // P
    n_col = d // fc
    seg = min(1024, fc)  # broadcast segment (psum tile = seg/512 banks)
    n_seg = fc // seg
    mm = min(512, seg)  # per-matmul free-dim limit

    nc = bacc.Bacc("TRN2", target_bir_lowering=False, debug=debug)

    # host-interleaved xnew: planes 0-2 = conv state, plane 3 = x
    xin_d = nc.dram_tensor("xin", [bpc, 4, d], fp32, kind="ExternalInput")
    wb_d = nc.dram_tensor("wb", [1, 5 * d], fp32, kind="ExternalInput")
    out_d = nc.dram_tensor("out", [bpc, d], fp32, kind="ExternalOutput")
    tail_d = nc.dram_tensor("tail", [bpc, 3, d], fp32, kind="ExternalOutput")

    with tile.TileContext(nc) as tc, ExitStack() as ctx:
        constp = ctx.enter_context(tc.tile_pool(name="const", bufs=1))
        iop = ctx.enter_context(tc.tile_pool(name="io", bufs=2))
        outp = ctx.enter_context(tc.tile_pool(name="outp", bufs=3))
        accp = ctx.enter_context(tc.tile_pool(name="acc", bufs=2))
        tmpp = ctx.enter_context(tc.tile_pool(name="tmp", bufs=1))
        stripp = ctx.enter_context(tc.tile_pool(name="strip", bufs=2))
        psump = ctx.enter_context(tc.tile_pool(name="psum", bufs=4, space="PSUM"))

        # --- broadcast weights+bias to all 128 partitions via PE ---
        # ones[1,128].T @ wb_strip[1,N] -> psum[128,N], then ACT-copy to SBUF.
        ones_t = constp.tile([1, P], fp32)
        nc.vector.memset(ones_t[0:1, :], 1.0)
        wbt = constp.tile([P, 5 * d], fp32)
        for c in range(n_col):
            for k in range(5):
                for s in range(n_seg):
                    off = k * d + c * fc + s * seg
                    strip = stripp.tile([1, seg], fp32, tag="strip")
                    nc.sync.dma_start(strip[0:1, :], wb_d[0:1, off : off + seg])
                    pt = psump.tile([P, seg], fp32, tag="bcast")
                    for m in range(seg // mm):
                        nc.tensor.matmul(
                            pt[:, m * mm : (m + 1) * mm],
                            ones_t[0:1, :],
                            strip[0:1, m * mm : (m + 1) * mm],
                            start=True,
                            stop=True,
                        )
                    nc.scalar.copy(wbt[:, off : off + seg], pt[:, :])

        def wsl(k, c):
            return wbt[:, k * d + c * fc : k * d + (c + 1) * fc]

        # --- main streaming loop ---
        # c-outer: the c=1 weight load hides behind the entire c=0 sweep.
        for c in range(n_col):
            cols = slice(c * fc, (c + 1) * fc)
            for r in range(n_row):
                rows = slice(r * P, (r + 1) * P)
                # xnew tile: planes 0-2 = conv state, plane 3 = x
                xs_t = iop.tile([P, 4, fc], fp32, tag="xs")
                nc.sync.dma_start(xs_t[:, :, :], xin_d[rows, :, cols])

                acc = accp.tile([P, fc], fp32, tag="acc")
                tmp = tmpp.tile([P, fc], fp32, tag="tmp")
                nc.vector.tensor_mul(acc[:, :], xs_t[:, 0, :], wsl(0, c))
                nc.vector.tensor_mul(tmp[:, :], xs_t[:, 1, :], wsl(1, c))
                nc.vector.tensor_add(acc[:, :], acc[:, :], tmp[:, :])
                nc.vector.tensor_mul(tmp[:, :], xs_t[:, 2, :], wsl(2, c))
                nc.vector.tensor_add(acc[:, :], acc[:, :], tmp[:, :])
                nc.vector.tensor_mul(tmp[:, :], xs_t[:, 3, :], wsl(3, c))
                nc.vector.tensor_add(acc[:, :], acc[:, :], tmp[:, :])
                nc.vector.tensor_add(acc[:, :], acc[:, :], wsl(4, c))

                out_t = outp.tile([P, fc], fp32, tag="out")
                nc.scalar.activation(
                    out_t[:, :], acc[:, :], mybir.ActivationFunctionType.Silu
                )
                nc.sync.dma_start(out_d[rows, cols], out_t[:, :])
                nc.sync.dma_start(tail_d[rows, :, cols], xs_t[:, 1:4, :])

    nc.compile()
    return nc


def _get_nc():
    global _NC
    if _NC is None:
        _NC = _build_nc()
    return _NC


def run_spmd(in_maps, trace=False):
    from concourse import bass_utils

    return bass_utils.run_bass_kernel_spmd(
        _get_nc(), in_maps, core_ids=list(range(N_CORES)), trace=trace
    )


def make_wb(weight, bias, d=D, fc=FC):
    return (
        np.concatenate([weight, bias[None, :]], axis=0)
        .astype(np.float32)
        .reshape(1, 5 * d)
    )


def make_in_maps(x, cs_rows, weight, bias):
    wb = make_wb(weight, bias)
    # interleave [cs0, cs1, cs2, x] once on host
    xin = np.concatenate(
        [np.asarray(cs_rows, dtype=np.float32), np.asarray(x, dtype=np.float32)],
        axis=1,
    )
    in_maps = []
    for c in range(N_CORES):
        sl = slice(c * BPC, (c + 1) * BPC)
        in_maps.append(
            {
                "xin": np.ascontiguousarray(xin[sl]),
                "wb": wb,
            }
        )
    return in_maps


def kernel(x, conv_state, weight, bias, conv_state_indices, _trace=False):
    x = np.asarray(x, dtype=np.float32)
    conv_state = np.asarray(conv_state, dtype=np.float32)
    weight = np.asarray(weight, dtype=np.float32)
    bias = np.asarray(bias, dtype=np.float32)
    idx = np.asarray(conv_state_indices, dtype=np.int32)

    fast = bool(np.array_equal(idx, np.arange(B, dtype=np.int32)))
    cs_rows = conv_state[:B] if fast else conv_state[idx]

    res = run_spmd(make_in_maps(x.reshape(B, S, D), cs_rows, weight, bias), trace=_trace)
    results = res.results

    out = np.concatenate([np.asarray(r["out"]) for r in results], axis=0)
    out = out.reshape(B, S, D)
    tails = np.concatenate([np.asarray(r["tail"]) for r in results], axis=0)

    new_state = conv_state.copy()
    if fast:
        new_state[:B] = tails
    else:
        new_state[idx] = tails

    if _trace:
        return (out, new_state), res
    return out, new_state
